# revision 58
# baseline (speedup 1.0000x reference)
"""Trainium2 Bass kernel for dual-stream cross/self attention (nn_Attention).

Reference semantics (per batch b):
  qkv_s = x_s @ Wqkv  -> q_s,k_s,v_s  [H=16 heads, N=577 tokens, d=64]
  stream s output head h attends with q_s and (k_s,v_s) if h<10 else (k_o,v_o)
  out_s = concat_heads @ Wproj + bproj

Sharding: batch (16) data-parallel over 8 cores, 2 batches/core; weights
replicated. Per core, 4 sequences (2 batches x 2 streams) are processed.

v2 design notes (vs the original baseline):
  - wv / wproj / bias are SBUF-resident (loaded once, outside the rep loop);
    only the 16 qk weight tiles stream per batch (prefetched 2 positions
    ahead of their consuming chips).
  - The whole rep is one flat pipeline: scores of head i+1 are emitted
    before AV of head i so the PE has independent work while ScalarE
    exponentiates, and gemm work (qk/v/proj units) is broken into
    1-psum-bank "chips" woven into the gaps between score j-tiles.  qk
    units of the same batch feed the first attention segment (head 2n's
    scores need only units n and n+8 — PE is strictly in-order, so every
    producer is emitted before its consumer).
  - PSUM rings are split by consumer class so Act-paced score tiles never
    block DVE/Act-drained gemm tiles: scores 2x[128,2,512], AV 1x, gemm
    2x[128,512].
  - Softmax normalization: the ones-column of v gives sumexp on psum row 64
    for free; per-head DVE reciprocals collect per pair, one DRAM round
    trip + two partition-broadcast DMAs build a [128,N] bf16 recip tile
    (emitted a pair late so waits are pre-satisfied), and one in-place DVE
    mult normalizes the pair inside the bf16 att tile.  Attention
    numerators are copied unnormalized psum->att so psum recycles fast;
    odd heads stage through SBUF + DMA for the partition shift.
  - Gemm psum drains ride ScalarE (activation Copy) between exps; DVE keeps
    the attention-side work.  A post-pass deletes back-to-back duplicate
    InstLdweights (~630 removals, ~58ns each on hardware).
"""

import numpy as np

import concourse.bass as bass
import concourse.mybir as mybir
import concourse.tile as tile
from concourse.bass_utils import run_bass_kernel_spmd

# ---------------------------------------------------------------------------
# Workaround: this walrus build rejects any instruction carrying >1 sem wait
# ("Too many sync wait commands").  Post-process the scheduled program and
# move excess waits onto single-wait NoOps inserted just before, on the same
# engine (engines execute their stream in order, so this is equivalent).
# ---------------------------------------------------------------------------


def split_excess_waits(nc, max_waits=1):
    cnt = 0
    for f in nc.m.functions:
        for blk in f.blocks:
            insts = blk.instructions
            need = any(
                inst.sync_info is not None
                and len(inst.sync_info.on_wait) > max_waits
                for inst in insts
            )
            if not need:
                continue
            newl = []
            for inst in insts:
                si = inst.sync_info
                if si is not None and len(si.on_wait) > max_waits:
                    waits = list(si.on_wait)
                    for w in waits[max_waits:]:
                        nop = mybir.InstNoOp(
                            name=f"wsplit_{cnt}",
                            engine=inst.engine,
                            ins=[],
                            outs=[],
                            sync_info=mybir.SyncInfo(on_wait=[w], on_update=[]),
                        )
                        cnt += 1
                        newl.append(nop)
                    si.on_wait = waits[:max_waits]
                newl.append(inst)
            blk.instructions = newl
    return cnt

# ---------------------------------------------------------------------------
# Post-pass: drop InstLdweights that reload the exact weights already loaded
# by the immediately-preceding InstLdweights on PE (only Matmult/NoOp may
# intervene).  The PE array keeps its stationary operand across matmuls, so
# the reload is pure overhead; ones carrying sync become NoOps instead.
# ---------------------------------------------------------------------------


def _ldw_sig(inst):
    try:
        a = inst.ins[0]
        return (
            str(a.memref),
            a.offset,
            str(a.ap),
            str(a.dtype),
            getattr(inst, "perf_mode", None),
            getattr(inst, "is_transpose", None),
            getattr(inst, "tile_position", None),
            getattr(inst, "tile_size", None),
        )
    except Exception:
        return None


def dedup_ldweights(nc):
    removed = 0
    for f in nc.m.functions:
        for blk in f.blocks:
            last_sig = None
            newl = []
            for inst in blk.instructions:
                eng = str(inst.engine).split(".")[-1]
                if eng != "PE":
                    newl.append(inst)
                    continue
                op = type(inst).__name__
                if op == "InstLdweights":
                    sig = _ldw_sig(inst)
                    if sig is not None and sig == last_sig:
                        si = inst.sync_info
                        if si is not None and (si.on_wait or si.on_update):
                            newl.append(
                                mybir.InstNoOp(
                                    name=f"ldwdup_{removed}",
                                    engine=inst.engine,
                                    ins=[],
                                    outs=[],
                                    sync_info=si,
                                )
                            )
                        removed += 1
                        continue
                    last_sig = sig
                    newl.append(inst)
                elif op in ("InstMatmult", "InstNoOp"):
                    newl.append(inst)
                else:
                    last_sig = None
                    newl.append(inst)
            blk.instructions = newl
    return removed


F32 = mybir.dt.float32

UNROLL = 1       # reps per For_i iteration (the loop barrier amortization
                 # turned out to lose to instruction-fetch pressure at 4)

N = 577          # tokens
C = 1024         # model dim
H = 16           # heads
D = 64           # head dim
HS = 10          # first HS heads self-attend, rest cross-attend
KT = 8           # c_in tiles of 128
SCALE = D ** -0.5
NCORES = 8
BL = 2           # local batches per core
NSEQ = 2 * BL    # sequences per core (batch-major, stream-minor)

# token partition tiles (start, len)
TOKT = [(0, 128), (128, 128), (256, 128), (384, 128), (512, 65)]
# token free-dim chunks (start, len): overlap 1 col at 288 so both are 289
# wide and a single ScalarE op can cover both PSUM sub-banks garbage-free
CH = [(0, 289), (288, 289)]


def build_kernel(cdt, reps=1, mode="full"):
    nc = bass.Bass()
    xt = nc.dram_tensor("xt", [NSEQ, C, N], cdt, kind="ExternalInput")
    wqkv = nc.dram_tensor("wqkv", [KT, 24, 128, 128], cdt, kind="ExternalInput")
    wproj = nc.dram_tensor("wproj", [KT, 8, 128, 128], cdt, kind="ExternalInput")
    biasr = nc.dram_tensor("biasr", [128, C], cdt, kind="ExternalInput")
    out = nc.dram_tensor("out", [NSEQ, N, C], F32, kind="ExternalOutput")

    import contextlib
    import itertools
    _uid = itertools.count()

    with tile.TileContext(nc) as tc:
        with (
            tc.tile_pool(name="const", bufs=1) as constp,
            tc.tile_pool(name="xa", bufs=4) as xap,       # xt + attnT share
            tc.tile_pool(name="qk", bufs=8) as qkp,       # q,k of 2 batches
            tc.tile_pool(name="vp", bufs=3) as vpp,
            tc.tile_pool(name="ep", bufs=2) as epp,
            tc.tile_pool(name="w1", bufs=24) as w1p,       # streamed qk weights
            tc.tile_pool(name="rbp", bufs=3) as rbpp,     # recip broadcast
            tc.tile_pool(name="rcp", bufs=2) as rcpp,     # recip collect rows
            tc.tile_pool(name="stg", bufs=2) as stgp,     # odd-head staging
            tc.tile_pool(name="op", bufs=2) as outp,
            tc.tile_pool(name="dr", bufs=4, space="DRAM") as drp,
            tc.tile_pool(name="ps", bufs=2, space="PSUM") as psp,
        ):
            bias_sb = constp.tile([128, C], cdt, tag="bias")
            nc.sync.dma_start(out=bias_sb[:], in_=biasr[:])
            wv_sb = constp.tile([128, KT, 8, 128], cdt, tag="wv")
            wp_sb = constp.tile([128, KT, 8, 128], cdt, tag="wp")
            for kk in range(KT):
                nc.sync.dma_start(
                    out=wv_sb[:, kk],
                    in_=wqkv[kk, 16:24].rearrange("n p f -> p n f"),
                )
                nc.sync.dma_start(
                    out=wp_sb[:, kk],
                    in_=wproj[kk].rearrange("n p f -> p n f"),
                )

            state = {}

            def load_xt(b):
                st = state.setdefault(b, {})
                st["xts"] = []
                for s in range(2):
                    t = xap.tile([128, KT, N], cdt, tag="xa", name=f"xt_{next(_uid)}")
                    nc.sync.dma_start(
                        out=t[:],
                        in_=xt[2 * b + s].rearrange("(kt p) n -> p kt n", p=128),
                    )
                    st["xts"].append(t)
                st["q"] = [
                    qkp.tile([128, 8, N], cdt, tag="qk", name=f"q_{next(_uid)}")
                    for s in range(2)
                ]
                st["k"] = [
                    qkp.tile([128, 8, N], cdt, tag="qk", name=f"k_{next(_uid)}")
                    for s in range(2)
                ]
                st["v"] = {}
                st["et"] = {}
                st["att"] = {}
                st["rcp"] = {}
                st["rd"] = {}
                st["rb"] = {}

            def ensure_v(b, s):
                st = state[b]
                if s not in st["v"]:
                    v = vpp.tile(
                        [128, 5, H, D + 1], cdt, tag="v", name=f"v_{next(_uid)}"
                    )
                    nc.vector.memset(v[:, :, :, D:D + 1], 1.0)
                    st["v"][s] = v

            # Gemm work is emitted in 1-psum-bank "chips" (8 matmuls + one
            # drain) so they can be woven between score j-tiles, covering the
            # PE stream while the Activation engine exponentiates.
            def qk_unit_chips(b, n):
                wts = []

                def prefetch():
                    for kk in range(KT):
                        w = w1p.tile([128, 128], cdt, tag="w1",
                                     name=f"wqk_{next(_uid)}")
                        nc.sync.dma_start(out=w[:], in_=wqkv[kk, n])
                        wts.append(w)

                def chip(s, ci):
                    st = state[b]
                    if not wts:
                        prefetch()
                    dst = st["q"][s] if n < 8 else st["k"][s]
                    nd = n % 8
                    c0, cl = CH[ci]
                    ps = psp.tile([128, 512], F32, tag="gm", bufs=2,
                                  name=f"ps_{next(_uid)}")
                    for kk in range(KT):
                        nc.tensor.matmul(
                            ps[:, 0:cl],
                            lhsT=wts[kk],
                            rhs=st["xts"][s][:, kk, c0:c0 + cl],
                            start=(kk == 0),
                            stop=(kk == KT - 1),
                        )
                    if ci == 0:
                        nc.scalar.copy(out=dst[:, nd, 0:289], in_=ps[:, 0:289])
                    else:
                        nc.scalar.copy(
                            out=dst[:, nd, 288:577], in_=ps[:, 0:289]
                        )

                return prefetch, [
                    (lambda s=s, ci=ci: chip(s, ci))
                    for s in range(2) for ci in range(2)
                ]

            def v_unit_chips(b, s, ti):
                t0, tl = TOKT[ti]

                def chip(ci):
                    st = state[b]
                    ensure_v(b, s)
                    ps = psp.tile([128, 512], F32, tag="gm", bufs=2,
                                  name=f"ps_{next(_uid)}")
                    for kk in range(KT):
                        nc.tensor.matmul(
                            ps[0:tl, :],
                            lhsT=st["xts"][s][:, kk, t0:t0 + tl],
                            rhs=wv_sb[:, kk, 4 * ci:4 * ci + 4, :],
                            start=(kk == 0),
                            stop=(kk == KT - 1),
                        )
                    nc.scalar.copy(
                        out=st["v"][s][0:tl, ti, 8 * ci:8 * ci + 8, 0:D],
                        in_=ps[0:tl, :].rearrange("p (h d) -> p h d", d=D),
                    )

                return [lambda ci=ci: chip(ci) for ci in range(2)]

            def emit_scores(b, s, h, gap=None):
                st = state[b]
                kv = s if h < HS else 1 - s
                par = (h % 2) * D
                nt = h // 2
                et = epp.tile(
                    [128, 5, 2, 289], cdt, tag="et", name=f"et_{next(_uid)}"
                )
                for jt, (j0, jl) in enumerate(TOKT):
                    ps = psp.tile(
                        [128, 2, 512], F32, tag="sc", name=f"ps_{next(_uid)}"
                    )
                    for ci, (c0, cl) in enumerate(CH):
                        nc.tensor.matmul(
                            ps[0:jl, ci, 0:cl],
                            lhsT=st["k"][kv][par:par + D, nt, j0:j0 + jl],
                            rhs=st["q"][s][par:par + D, nt, c0:c0 + cl],
                            start=True,
                            stop=True,
                        )
                    nc.scalar.activation(
                        out=et[0:jl, jt],
                        in_=ps[0:jl, :, 0:289],
                        func=mybir.ActivationFunctionType.Exp,
                        scale=SCALE,
                    )
                    if gap is not None:
                        gap()   # weave one gemm chip between score j-tiles
                st["et"][(s, h)] = et

            def emit_av(b, s, h):
                st = state[b]
                ensure_v(b, s)
                ensure_v(b, 1 - s)
                kv = s if h < HS else 1 - s
                hh = h % 2
                nt = h // 2
                att = st["att"][s]
                et = st["et"].pop((s, h))

                pa = psp.tile([128, 2, 512], F32, tag="pa", bufs=1,
                              name=f"pa_{next(_uid)}")
                for jt, (j0, jl) in enumerate(TOKT):
                    for ci in range(2):
                        nc.tensor.matmul(
                            pa[0:D + 1, ci, 0:289],
                            lhsT=st["v"][kv][0:jl, jt, h, :],
                            rhs=et[0:jl, jt, ci],
                            start=(jt == 0),
                            stop=(jt == 4),
                        )

                if hh == 0:
                    rcp = rcpp.tile([D + 1, 2, N], cdt, tag="rcp",
                                    name=f"rcp_{next(_uid)}")
                    st["rcp"][nt] = rcp
                else:
                    rcp = st["rcp"][nt]
                with nc.allow_low_precision(
                    reason="per-token softmax scale; bf16 is ample"
                ):
                    nc.vector.reciprocal(
                        out=rcp[D:D + 1, hh, 0:289], in_=pa[D:D + 1, 0, 0:289]
                    )
                    nc.vector.reciprocal(
                        out=rcp[D:D + 1, hh, 288:577], in_=pa[D:D + 1, 1, 0:289]
                    )

                if hh == 0:
                    nc.vector.tensor_copy(
                        out=att[0:D, nt, 0:289], in_=pa[0:D, 0, 0:289]
                    )
                    nc.vector.tensor_copy(
                        out=att[0:D, nt, 289:577], in_=pa[0:D, 1, 1:289]
                    )
                else:
                    stg = stgp.tile([D, N], cdt, tag="stg", name=f"st_{next(_uid)}")
                    nc.vector.tensor_copy(out=stg[:, 0:289], in_=pa[0:D, 0, 0:289])
                    nc.vector.tensor_copy(out=stg[:, 289:577], in_=pa[0:D, 1, 1:289])
                    nc.sync.dma_start(out=att[D:128, nt, :], in_=stg[:])

                    # pair complete: write both recip rows to DRAM; the
                    # partition-broadcast read back is emitted a pair later
                    # (emit_bcast) so its wait is satisfied by issue time.
                    rd = drp.tile([2 * N], cdt, tag="rd",
                                  name=f"rd_{next(_uid)}")
                    nc.scalar.dma_start(
                        out=rd[None, :], in_=rcp[D:D + 1, :, :]
                    )
                    st["rd"][nt] = rd

            def emit_bcast(b, s, nt):
                st = state[b]
                rd = st["rd"].pop(nt)
                rb = rbpp.tile([128, N], cdt, tag="rb",
                               name=f"rb_{next(_uid)}")
                nc.scalar.dma_start(
                    out=rb[0:D, :],
                    in_=rd[None, 0:N].to_broadcast([D, N]),
                )
                nc.scalar.dma_start(
                    out=rb[D:128, :],
                    in_=rd[None, N:2 * N].to_broadcast([D, N]),
                )
                st["rb"][nt] = rb

            def emit_norm(b, s, nt):
                st = state[b]
                att = st["att"][s]
                rb = st["rb"].pop(nt)
                nc.gpsimd.tensor_tensor(
                    out=att[:, nt, :],
                    in0=att[:, nt, :],
                    in1=rb[:],
                    op=mybir.AluOpType.mult,
                )

            def proj_unit_chips(b, s, ti):
                t0, tl = TOKT[ti]

                def chip(ci):
                    st = state[b]
                    att = st["att"][s]
                    ps = psp.tile([128, 512], F32, tag="gm", bufs=2,
                                  name=f"ps_{next(_uid)}")
                    for kk in range(KT):
                        nc.tensor.matmul(
                            ps[0:tl, :],
                            lhsT=att[:, kk, t0:t0 + tl],
                            rhs=wp_sb[:, kk, 4 * ci:4 * ci + 4, :],
                            start=(kk == 0),
                            stop=(kk == KT - 1),
                        )
                    ob = outp.tile([128, 512], F32, tag="ob",
                                   name=f"ob_{next(_uid)}")
                    nc.vector.tensor_tensor(
                        out=ob[0:tl, :],
                        in0=ps[0:tl, :],
                        in1=bias_sb[0:tl, 512 * ci:512 * ci + 512],
                        op=mybir.AluOpType.add,
                    )
                    nc.sync.dma_start(
                        out=out[2 * b + s, t0:t0 + tl, 512 * ci:512 * ci + 512],
                        in_=ob[0:tl, :],
                    )

                return [lambda ci=ci: chip(ci) for ci in range(2)]

            def emit_rep():
                # ---- build the flat unit sequence for one rep ----
                load_xt(0)

                # heads in execution order + per-position feeder chip lists
                heads = []
                feeders = {}   # head index -> list of (kind, payload)

                def FC(i, chips):
                    feeders.setdefault(i, []).extend(
                        ("chip", c) for c in chips
                    )

                def FX(i, th):
                    feeders.setdefault(i, []).append(("now", th))

                for s in range(2):
                    for h in range(H):
                        heads.append((0, s, h))
                if mode != "qkv":
                    for s in range(2):
                        for h in range(H):
                            heads.append((1, s, h))

                # SEG0 (att b0s0, i=0..15): qk(b0) units woven just ahead of
                # the scores that need them.  Head h=2nt's scores (emitted at
                # loop index 2nt-1 as the lookahead) need units nt and nt+8
                # fully emitted first -- PE executes in order, so a
                # later-emitted producer would deadlock.  Chips queued at
                # index p are force-drained at the start of iteration p+1,
                # i.e. before the scores of head p+2.  Units (0,8),(1,9) +
                # v(b0,s0) + v(b0,s1,ti=0) are emitted before the pipeline
                # starts; the rest weave here.  v(b0,s1) is first needed by
                # the cross AV of head 10 at i=10.
                preq = []
                for j, nt in enumerate(range(2, 8)):
                    pos = (0, 1, 3, 5, 7, 9)[j]
                    for n in (nt, nt + 8):
                        pf, chips = qk_unit_chips(0, n)
                        if pos < 2:
                            preq.append(pf)
                        else:
                            FX(pos - 2, pf)
                        FC(pos, chips)
                for j in range(4):
                    FC(2 + 2 * j, v_unit_chips(0, 1, j + 1))
                # load b1 inputs once xt00/xt01 consumers are done
                FX(10, lambda: load_xt(1))
                # SEG1 (att b0s1, i=16..31): 16 qk(b1) + 5 v(b1,s0)
                for j in range(16):
                    pf, chips = qk_unit_chips(1, j)
                    FX(14 + j, pf)
                    FC(16 + j, chips)
                for j in range(5):
                    FC(17 + 3 * j, v_unit_chips(1, 0, j))
                if mode != "qkv":
                    # SEG2 (att b1s0, i=32..47): 5 v(b1,s1) early, 5 proj(b0,s0)
                    for j in range(5):
                        FC(32 + j, v_unit_chips(1, 1, j))
                if mode == "full":
                    for j in range(5):
                        FC(38 + 2 * j, proj_unit_chips(0, 0, j))
                    for j in range(5):
                        FC(48 + 2 * j, proj_unit_chips(0, 1, j))
                        FC(51 + 2 * j, proj_unit_chips(1, 0, j))

                # ---- run the pipeline ----
                if mode == "qkv":
                    for bb in range(2):
                        if bb == 1:
                            load_xt(1)
                        for n in range(16):
                            pf, chips = qk_unit_chips(bb, n)
                            pf()
                            for c in chips:
                                c()
                    for b in range(2):
                        for s in range(2):
                            for ti in range(5):
                                for c in v_unit_chips(b, s, ti):
                                    c()
                    state.clear()
                else:
                    # head-0/1 producers must precede the pipelined scores;
                    # weight prefetches interleave so at most 3 un-consumed
                    # units are in the w1 ring (24 tiles) at once.
                    us = {n: qk_unit_chips(0, n) for n in (0, 8, 1, 9)}
                    us[0][0]()
                    us[8][0]()
                    us[1][0]()
                    for c in us[0][1]:
                        c()
                    us[9][0]()
                    for c in us[8][1]:
                        c()
                    preq[0]()
                    for c in us[1][1]:
                        c()
                    preq[1]()
                    for c in us[9][1]:
                        c()
                    for ti in range(5):
                        for c in v_unit_chips(0, 0, ti):
                            c()
                    preq[2]()
                    for c in v_unit_chips(0, 1, 0):
                        c()
                    preq[3]()

                    pend = []   # (due_index, thunk)
                    gmq = []    # (queued_index, chip)

                    def gap():
                        if gmq:
                            gmq.pop(0)[1]()

                    def alloc_att(b, s):
                        st = state[b]
                        if s not in st["att"]:
                            st["att"][s] = xap.tile(
                                [128, KT, N], cdt, tag="xa",
                                name=f"att_{next(_uid)}",
                            )

                    b0, s0, h0 = heads[0]
                    alloc_att(b0, s0)
                    emit_scores(b0, s0, h0)
                    for i, (b, s, h) in enumerate(heads):
                        # chips queued 2+ positions ago must be emitted before
                        # the scores of head i+1 (producer ordering); chips
                        # queued at i-1 stay for the score-gap weave.
                        while gmq and gmq[0][0] < i - 1:
                            gmq.pop(0)[1]()
                        if i + 1 < len(heads):
                            bn, sn, hn = heads[i + 1]
                            if sn not in state[bn]["att"]:
                                alloc_att(bn, sn)
                            emit_scores(bn, sn, hn, gap=gap)
                        emit_av(b, s, h)
                        if h % 2 == 1:
                            nt = h // 2
                            pend.append((i + 2, lambda b=b, s=s, nt=nt:
                                         emit_bcast(b, s, nt)))
                            pend.append((i + 4, lambda b=b, s=s, nt=nt:
                                         emit_norm(b, s, nt)))
                        while pend and pend[0][0] <= i:
                            pend.pop(0)[1]()
                        for kind, pl in feeders.get(i, []):
                            if kind == "now":
                                pl()
                            else:
                                gmq.append((i, pl))
                    while gmq:
                        gmq.pop(0)[1]()
                    while pend:
                        pend.pop(0)[1]()
                    if mode == "full":
                        for ti in range(5):
                            for c in proj_unit_chips(1, 1, ti):
                                c()
                    state.clear()

            # For_i puts an all-engine barrier at every back edge, killing
            # cross-rep overlap of the rep head (xt load + qkv preamble) with
            # the previous tail.  Unroll several reps per iteration so the
            # barrier amortizes and adjacent reps pipeline via the tile rings.
            if reps > 1:
                unroll = UNROLL if reps % UNROLL == 0 else 1
                with tc.For_i(0, reps // unroll, 1):
                    for _ in range(unroll):
                        emit_rep()
            else:
                emit_rep()

    dedup_ldweights(nc)
    split_excess_waits(nc)
    return nc


_CACHE = {}

CDT = mybir.dt.bfloat16  # compute dtype knob: bfloat16 | float32r | float32


def _get_nc(reps=1, mode="full"):
    key = (str(CDT), reps, mode)
    if key not in _CACHE:
        _CACHE[key] = build_kernel(CDT, reps=reps, mode=mode)
    return _CACHE[key]


def prep_in_maps(x1, x2, Wqkv, Wproj, bproj, cdt=None):
    cdt = cdt or CDT
    np_cdt = mybir.dt.np(cdt)
    x1 = np.asarray(x1, dtype=np.float32)
    x2 = np.asarray(x2, dtype=np.float32)
    Wqkv = np.asarray(Wqkv, dtype=np.float32)
    Wproj = np.asarray(Wproj, dtype=np.float32)
    bproj = np.asarray(bproj, dtype=np.float32)

    wq = np.ascontiguousarray(
        Wqkv.reshape(KT, 128, 24, 128).transpose(0, 2, 1, 3)
    ).astype(np_cdt)
    wp = np.ascontiguousarray(
        Wproj.reshape(KT, 128, 8, 128).transpose(0, 2, 1, 3)
    ).astype(np_cdt)
    biasr = np.ascontiguousarray(
        np.broadcast_to(bproj, (128, C))
    ).astype(np_cdt)

    # [B, N, C] -> per-core [NSEQ, C, N], batch-major stream-minor
    xt_all = np.empty((NCORES, NSEQ, C, N), dtype=np_cdt)
    for c in range(NCORES):
        for lb in range(BL):
            b = BL * c + lb
            xt_all[c, 2 * lb + 0] = x1[b].T.astype(np_cdt)
            xt_all[c, 2 * lb + 1] = x2[b].T.astype(np_cdt)

    return [
        {"xt": xt_all[c], "wqkv": wq, "wproj": wp, "biasr": biasr}
        for c in range(NCORES)
    ]


def unpack_results(results):
    out1 = np.empty((NCORES * BL, N, C), dtype=np.float32)
    out2 = np.empty((NCORES * BL, N, C), dtype=np.float32)
    for c in range(NCORES):
        o = results[c]["out"]
        for lb in range(BL):
            out1[BL * c + lb] = o[2 * lb + 0]
            out2[BL * c + lb] = o[2 * lb + 1]
    return out1, out2


def kernel(x1, x2, Wqkv, Wproj, bproj):
    nc = _get_nc()
    in_maps = prep_in_maps(x1, x2, Wqkv, Wproj, bproj)
    res = run_bass_kernel_spmd(nc, in_maps, core_ids=list(range(NCORES)))
    return unpack_results(res.results)
